# revision 19
# baseline (speedup 1.0000x reference)
"""Trainium2 Bass kernel for nn_CrossAttention (T5-style cross attention
with relative position bias), sharded over 8 NeuronCores.

Sharding: core c handles batch b = c//4 and heads [4*(c%4), 4*(c%4)+4).
Each core computes q/k/v projections for its heads and attention with the
relative-position bias. The normalized per-head attention outputs are then
exchanged within each 4-core group via a masked zero-padded bf16
ReduceScatter (2x 1.5MB wire instead of the 8MB fp32 output ReduceScatter),
after which every core holds all 16 heads for its 512-query slice and runs
the full output projection locally.

Perf-relevant structure vs the original baseline:
- All inputs/weights are converted to bf16 host-side: halves HBM traffic
  and avoids fp32-HIGH matmul mode.
- Bias banks are built with 4 overlapping-window DMAs (one per head)
  instead of 512 per-row DMAs that previously serialized ~150us on the
  scalar/gpsimd engines and let the PE clock-gate down.
- exp() runs on [128,1024] tiles spanning two PSUM banks: fewer ACTIVATE
  instructions (the scalar engine is the attention-phase roofline).
- Softmax row sums come from the ones-column trick in the V stationary
  operand; 1/z uses a DMA round-trip reshape to [128,8] and one fast
  approx-reciprocal instead of a 3.4us single-partition RECIPROCAL.
- A single shared PSUM layout (3x [128,1024] + 2x [128,512] banks) is used
  by every phase so projection, attention and output stages can overlap.
"""
import os
import numpy as np

import concourse.bass as bass
import concourse.mybir as mybir
import concourse.tile as tile
from concourse import bacc
from concourse.bass_utils import run_bass_kernel_spmd

dt = mybir.dt
AF = mybir.ActivationFunctionType

B, S, K, E, H, D = 2, 2048, 2048, 1024, 16, 64
NB, MAXD = 32, 128
HL = 4            # heads per core
NP = 2            # head pairs per core
SB = 512          # s block
NSB = S // SB     # 4
ET = E // 128     # 8 contraction tiles
JT = K // 128     # 16 key tiles
KB = K // SB      # 4 key blocks
BANKW = 3968      # bias bank free width

_prog = None


def _bucket1d():
    # T5 bidirectional bucket over rel = kk - s in [-2047, 2047].
    r = np.arange(-(K - 1), K)
    nb = NB // 2
    buckets = (r > 0).astype(np.int64) * nb
    a = np.abs(r)
    max_exact = nb // 2
    rf = np.maximum(a, 1).astype(np.float32)
    large = max_exact + (
        np.log(rf / max_exact) / np.log(MAXD / max_exact) * (nb - max_exact)
    ).astype(np.int64)
    large = np.minimum(large, nb - 1)
    return buckets + np.where(a < max_exact, a, large)


def _runs_rev():
    rev = _bucket1d()[::-1]  # x = 0..4094  <->  rel = 2047 - x
    runs, start = [], 0
    for x in range(1, len(rev)):
        if rev[x] != rev[start]:
            runs.append((start, x - start, int(rev[start])))
            start = x
    runs.append((start, len(rev) - start, int(rev[start])))
    return runs


def _build():
    nc = bacc.Bacc("TRN2", target_bir_lowering=False, debug=False, num_devices=8)
    f32, bf16 = dt.float32, dt.bfloat16

    hsT = nc.dram_tensor("hsT", [E, S], bf16, kind="ExternalInput")
    kvT = nc.dram_tensor("kvT", [E, K], bf16, kind="ExternalInput")
    wq = nc.dram_tensor("wq", [E, HL * D], bf16, kind="ExternalInput")
    wk = nc.dram_tensor("wk", [E, HL * D], bf16, kind="ExternalInput")
    wv = nc.dram_tensor("wv", [E, HL * D], bf16, kind="ExternalInput")
    wo = nc.dram_tensor("wo", [E, E], bf16, kind="ExternalInput")
    rbT = nc.dram_tensor("rbT", [HL, NB], f32, kind="ExternalInput")
    msk = nc.dram_tensor("msk", [128, 4 * SB], bf16, kind="ExternalInput")
    out_part = nc.dram_tensor("out_part", [SB, E], f32, kind="ExternalOutput")

    runs = _runs_rev()
    maxrun = max(ln for _, ln, _ in runs)
    groups = [[0, 1, 2, 3], [4, 5, 6, 7]]

    with tile.TileContext(nc) as tc:
        with (
            tc.tile_pool(name="wpool", bufs=1) as wpool,
            tc.tile_pool(name="bigpool", bufs=1) as bigpool,
            tc.tile_pool(name="dram", bufs=1, space="DRAM") as dram,
            tc.tile_pool(name="pspool", bufs=3, space="PSUM") as pspool,
            tc.tile_pool(name="opool", bufs=1, space="PSUM") as opool,
            tc.tile_pool(name="xpool", bufs=2) as xpool,
            tc.tile_pool(name="probs", bufs=4) as probs,
            tc.tile_pool(name="npool", bufs=3) as npool,
            tc.tile_pool(name="zdram", bufs=2, space="DRAM") as zdram,
            tc.tile_pool(name="outp", bufs=2) as outp,
            tc.tile_pool(name="obep", bufs=1) as obep,
        ):
            # ---------- weights ----------
            wq_sb = wpool.tile([128, ET, HL * D], bf16)
            nc.sync.dma_start(wq_sb[:], wq.ap().rearrange("(et p) m -> p et m", p=128))
            wk_sb = wpool.tile([128, ET, HL * D], bf16)
            nc.sync.dma_start(wk_sb[:], wk.ap().rearrange("(et p) m -> p et m", p=128))
            wv_sb = wpool.tile([128, ET, HL * D], bf16)
            nc.sync.dma_start(wv_sb[:], wv.ap().rearrange("(et p) m -> p et m", p=128))
            wo_sb = wpool.tile([128, 8, E], bf16)
            nc.gpsimd.dma_start(wo_sb[:], wo.ap().rearrange("(g p) e -> p g e", p=128))
            rbT_sb = wpool.tile([HL, NB], f32)
            nc.sync.dma_start(rbT_sb[:], rbT[:])
            msk_sb = wpool.tile([128, 4 * SB], bf16)
            nc.gpsimd.dma_start(msk_sb[:], msk[:])

            # ---------- bias banks ----------
            ones = wpool.tile([HL, maxrun], f32)
            nc.vector.memset(ones[:], 1.0)
            ed = wpool.tile([HL, 2 * K - 1], bf16)
            for st, ln, bk in runs:
                nc.scalar.activation(
                    ed[:, st : st + ln], ones[:, 0:ln], AF.Exp,
                    scale=rbT_sb[:, bk : bk + 1],
                )
            ed_dram = dram.tile([HL, 2 * K - 1], bf16)
            nc.gpsimd.dma_start(ed_dram[:], ed[:])
            banks = []
            for h in range(HL):
                bank_t = bigpool.tile([128, BANKW], bf16, tag=f"bank{h}")
                row = ed_dram[h : h + 1, :]
                src = bass.AP(row.tensor, row.offset, [[1, 128], [1, BANKW]])
                eng = nc.scalar if h % 2 == 0 else nc.gpsimd
                eng.dma_start(bank_t[:], src)
                banks.append(bank_t)

            # ---------- persistent activations ----------
            qT_sb, kT_sb = [], []
            for pr in range(NP):
                t_q = bigpool.tile([128, S], bf16, tag=f"qT{pr}")
                qT_sb.append(t_q)
                t_k = bigpool.tile([128, K], bf16, tag=f"kT{pr}")
                kT_sb.append(t_k)
            v_aug = []
            for h in range(HL):
                t_v = bigpool.tile([128, JT * 128], bf16, tag=f"vaug{h}")
                nc.vector.memset(t_v[:], 0.0)
                onescol = 64 if h % 2 == 0 else 32
                for jt in range(JT):
                    nc.vector.memset(t_v[:, jt * 128 + onescol : jt * 128 + onescol + 1], 1.0)
                v_aug.append(t_v)

            # ---------- KV projection ----------
            # pv accumulation groups sharing a PSUM bank must run
            # sequentially: a start=True matmul clears has_written for the
            # whole bank, so interleaved groups in one bank lose terms.
            kvT_r = kvT.ap().rearrange("(et p) j -> p et j", p=128)
            for kb in range(KB):
                ps_kv = pspool.tile([128, 1024], f32, tag="ps")
                pv0 = opool.tile([128, SB], f32, tag="o0")
                pv1 = opool.tile([128, SB], f32, tag="o1")
                kvts = []
                for et in range(ET):
                    kvt = xpool.tile([128, SB], bf16, tag=f"xt{et}", name=f"kvt{et}")
                    nc.sync.dma_start(kvt[:], kvT_r[:, et, kb * SB : (kb + 1) * SB])
                    kvts.append(kvt)
                    for pr in range(NP):
                        nc.tensor.matmul(
                            ps_kv[:, pr * SB : (pr + 1) * SB],
                            wk_sb[:, et, pr * 128 : (pr + 1) * 128],
                            kvt[:],
                            start=(et == 0), stop=(et == ET - 1),
                        )
                for kt in range(4):
                    pv = (pv0, pv1)[kt // 2]
                    for et in range(ET):
                        nc.tensor.matmul(
                            pv[:, (kt % 2) * 256 : (kt % 2) * 256 + 256],
                            kvts[et][:, kt * 128 : (kt + 1) * 128],
                            wv_sb[:, et, :],
                            start=(et == 0), stop=(et == ET - 1),
                        )
                    jt = kb * 4 + kt
                    for h in range(HL):
                        col0 = 0 if h % 2 == 0 else 64
                        nc.vector.tensor_copy(
                            v_aug[h][:, jt * 128 + col0 : jt * 128 + col0 + 64],
                            pv[:, (kt % 2) * 256 + h * D : (kt % 2) * 256 + (h + 1) * D],
                        )
                for pr in range(NP):
                    nc.vector.tensor_copy(
                        kT_sb[pr][:, kb * SB : (kb + 1) * SB],
                        ps_kv[:, pr * SB : (pr + 1) * SB],
                    )

            # ---------- Q projection ----------
            hsT_r = hsT.ap().rearrange("(et p) s -> p et s", p=128)
            for sb in range(NSB):
                ps_q = pspool.tile([128, 1024], f32, tag="ps")
                for et in range(ET):
                    hst = xpool.tile([128, SB], bf16, tag=f"xt{et}", name=f"hst{et}")
                    nc.gpsimd.dma_start(hst[:], hsT_r[:, et, sb * SB : (sb + 1) * SB])
                    for pr in range(NP):
                        nc.tensor.matmul(
                            ps_q[:, pr * SB : (pr + 1) * SB],
                            wq_sb[:, et, pr * 128 : (pr + 1) * 128],
                            hst[:],
                            start=(et == 0), stop=(et == ET - 1),
                        )
                for pr in range(NP):
                    nc.vector.tensor_copy(
                        qT_sb[pr][:, sb * SB : (sb + 1) * SB],
                        ps_q[:, pr * SB : (pr + 1) * SB],
                    )

            # ---------- attention ----------
            rs_in = [
                dram.tile([NSB, 4 * 128, SB], bf16, tag=f"rsin{pr}", name=f"rsin{pr}")
                for pr in range(NP)
            ]
            rs_out = [
                dram.tile([4 * 128, SB], bf16, tag=f"rsout{pr}", name=f"rsout{pr}")
                for pr in range(NP)
            ]
            for pr in range(NP):
                for sb in range(NSB):
                    po = [
                        opool.tile([128, SB], f32, tag=f"o{hh}", name=f"po{hh}")
                        for hh in range(2)
                    ]

                    def emit_qk(jtb):
                        ps_pair = []
                        for hh in range(2):
                            ps_t = pspool.tile(
                                [128, 1024], f32, tag="ps", name=f"ps{hh}"
                            )
                            ps_pair.append(ps_t)
                            for half in range(2):
                                jt = 2 * jtb + half
                                nc.tensor.matmul(
                                    ps_t[:, half * SB : (half + 1) * SB],
                                    kT_sb[pr][hh * 64 : (hh + 1) * 64, jt * 128 : (jt + 1) * 128],
                                    qT_sb[pr][hh * 64 : (hh + 1) * 64, sb * SB : (sb + 1) * SB],
                                    start=True, stop=True, tile_position=(hh * 64, 0),
                                )
                        return ps_pair

                    # QK for batch jtb+1 is emitted before AV of batch jtb so
                    # the PE queue never drains while exp/mul are in flight.
                    cur = emit_qk(0)
                    for jtb in range(JT // 2):
                        nxt = emit_qk(jtb + 1) if jtb < JT // 2 - 1 else None
                        for hh in range(2):
                            h = pr * 2 + hh
                            pb = probs.tile([128, 1024], bf16, tag="pb")
                            nc.scalar.activation(pb[:], cur[hh][:], AF.Exp)
                            for half in range(2):
                                jt = 2 * jtb + half
                                off = jt * 128 + sb * SB
                                # split the bias muls across DVE and GpSimd
                                # so neither gates the AV matmuls
                                meng = nc.vector if half == 0 else nc.gpsimd
                                meng.tensor_mul(
                                    pb[:, half * SB : (half + 1) * SB],
                                    pb[:, half * SB : (half + 1) * SB],
                                    banks[h][:, off : off + SB],
                                )
                                nc.tensor.matmul(
                                    po[hh][:],
                                    v_aug[h][:, jt * 128 : (jt + 1) * 128],
                                    pb[:, half * SB : (half + 1) * SB],
                                    start=(jt == 0), stop=(jt == JT - 1),
                                )
                        cur = nxt
                    # ---- park po in SBUF (frees the PSUM banks fast) ----
                    pair = npool.tile([128, SB], f32, tag="pair")
                    zrow = npool.tile([128, SB], f32, tag="zrow")
                    for hh in range(2):
                        h = pr * 2 + hh
                        zp = 64 if h % 2 == 0 else 32
                        ar = 0 if h % 2 == 0 else 64
                        nc.vector.tensor_copy(
                            pair[ar : ar + 64, :], po[hh][ar : ar + 64, :]
                        )
                        nc.vector.tensor_copy(
                            zrow[zp : zp + 1, :], po[hh][zp : zp + 1, :]
                        )
                    # ---- per-sb normalize + masked scatter ----
                    zd = zdram.tile([2, SB], f32, tag="zd", name="zd")
                    nc.sync.dma_start(zd[0:1, :], zrow[64:65, :])
                    nc.sync.dma_start(zd[1:2, :], zrow[32:33, :])
                    zr128 = npool.tile([128, 8], f32, tag="zr128")
                    zd0 = zd[0:1, :]
                    nc.sync.dma_start(
                        zr128[:], bass.AP(zd0.tensor, zd0.offset, [[8, 128], [1, 8]])
                    )
                    rz128 = npool.tile([128, 8], f32, tag="rz128")
                    nc.vector.reciprocal_approx_fast(rz128[:], zr128[:])
                    rzd = zdram.tile([2, SB], f32, tag="rzd", name="rzd")
                    rzd0 = rzd[0:1, :]
                    nc.sync.dma_start(
                        bass.AP(rzd0.tensor, rzd0.offset, [[8, 128], [1, 8]]), rz128[:]
                    )
                    zb = npool.tile([128, SB], f32, tag="zb")
                    for hh in range(2):
                        h = pr * 2 + hh
                        ar = 0 if h % 2 == 0 else 64
                        rr = rzd[hh : hh + 1, :]
                        nc.sync.dma_start(
                            zb[ar : ar + 64, :],
                            bass.AP(rr.tensor, rr.offset, [[0, 64], [1, SB]]),
                        )
                    chunk = npool.tile([128, SB], bf16, tag="chunk")
                    nc.vector.tensor_mul(chunk[:], pair[:], zb[:])
                    for t in range(4):
                        ct = npool.tile([128, SB], bf16, tag="ct")
                        nc.vector.tensor_mul(
                            ct[:], chunk[:], msk_sb[:, t * SB : (t + 1) * SB]
                        )
                        nc.sync.dma_start(
                            rs_in[pr][sb, t * 128 : (t + 1) * 128, :], ct[:]
                        )
                nc.gpsimd.collective_compute(
                    "ReduceScatter",
                    mybir.AluOpType.add,
                    replica_groups=groups,
                    ins=[rs_in[pr][:]],
                    outs=[rs_out[pr][:]],
                )
                # gath reads for this pr ride the gpsimd queue right after
                # the trigger: pr0's wait overlaps pr1's attention, and no
                # compute-feeding queue blocks behind a collective.
                if pr == 0:
                    gath = bigpool.tile([128, 8, SB], bf16)
                for j in range(4):
                    nc.gpsimd.dma_start(
                        gath[:, 2 * j + pr, :],
                        rs_out[pr][128 * j : 128 * (j + 1), :],
                    )

            # ---------- final output projection ----------
            # Pass 1: even dim-blocks (from RS#0) for ALL output tiles run
            # while RS#1 is in flight; pass 2 adds the odd blocks. Keeping
            # every odd-group matmul after every even group means no RS#1-
            # dependent matmul ever blocks a ready one in the PE queue.
            obe_all = {}
            for i in range(NSB):
                for ec in range(2):
                    pp = pspool.tile([128, 1024], f32, tag="ps", name="ppe")
                    for gi, g in enumerate([0, 2, 4, 6]):
                        nc.tensor.matmul(
                            pp[:, 0:SB],
                            gath[:, g, i * 128 : (i + 1) * 128],
                            wo_sb[:, g, ec * SB : (ec + 1) * SB],
                            start=(gi == 0), stop=(gi == 3),
                        )
                    obe = obep.tile(
                        [128, SB], f32, tag=f"obe{i}{ec}", name=f"obe{i}{ec}"
                    )
                    nc.vector.tensor_copy(obe[:], pp[:, 0:SB])
                    obe_all[(i, ec)] = obe
            for i in range(NSB):
                for ec in range(2):
                    pp = pspool.tile([128, 1024], f32, tag="ps", name="ppo")
                    for gi, g in enumerate([1, 3, 5, 7]):
                        nc.tensor.matmul(
                            pp[:, 0:SB],
                            gath[:, g, i * 128 : (i + 1) * 128],
                            wo_sb[:, g, ec * SB : (ec + 1) * SB],
                            start=(gi == 0), stop=(gi == 3),
                        )
                    ob = outp.tile([128, SB], f32, tag="ob")
                    nc.vector.tensor_add(ob[:], obe_all[(i, ec)][:], pp[:, 0:SB])
                    nc.sync.dma_start(
                        out_part[i * 128 : (i + 1) * 128, ec * SB : (ec + 1) * SB],
                        ob[:],
                    )

    nc.compile()
    return nc


def _get_prog():
    global _prog
    if _prog is None:
        _prog = _build()
    return _prog


def kernel(hidden_states, key_value_states, Wq, Wkv, Wo, rel_bias):
    bf = dt.np(dt.bfloat16)
    hidden_states = np.asarray(hidden_states, dtype=np.float32)
    key_value_states = np.asarray(key_value_states, dtype=np.float32)
    Wq = np.asarray(Wq, dtype=np.float32)
    Wkv = np.asarray(Wkv, dtype=np.float32)
    Wo = np.asarray(Wo, dtype=np.float32)
    rel_bias = np.asarray(rel_bias, dtype=np.float32)

    nc = _get_prog()
    wo_full = np.ascontiguousarray(Wo.astype(bf))
    in_maps = []
    for c in range(8):
        b = c // 4
        r = c % 4
        h0 = 4 * r                 # global head base
        cs, ce = h0 * D, h0 * D + HL * D
        msk_np = np.zeros([128, 4 * SB], dtype=bf)
        msk_np[:, r * SB : (r + 1) * SB] = 1
        in_maps.append(
            {
                "hsT": np.ascontiguousarray(hidden_states[b].T.astype(bf)),
                "kvT": np.ascontiguousarray(key_value_states[b].T[:, ::-1].astype(bf)),
                "wq": np.ascontiguousarray(Wq[:, cs:ce].astype(bf)),
                "wk": np.ascontiguousarray(Wkv[:, cs:ce].astype(bf)),
                "wv": np.ascontiguousarray(Wkv[:, E + cs : E + ce].astype(bf)),
                "wo": wo_full,
                "rbT": np.ascontiguousarray(rel_bias[:, h0 : h0 + HL].T),
                "msk": msk_np,
            }
        )

    trace = os.environ.get("KERNEL_TRACE", "0") == "1"
    r = run_bass_kernel_spmd(nc, in_maps, list(range(8)), trace=trace)
    if trace:
        print(f"HW exec time: {r.exec_time_ns} ns")
        kernel.last_result = r

    out = np.empty([B, S, E], dtype=np.float32)
    for c in range(8):
        b, rank = c // 4, c % 4
        out[b, rank * SB : (rank + 1) * SB] = r.results[c]["out_part"]
    return out
